# revision 5
# baseline (speedup 1.0000x reference)
"""Trainium2 Bass kernel for nn_ClosingPricePredictor (v2).

LSTM (N=512 batch, L=512 steps, I=64 in, H=1024 hidden) + 2-layer MLP head.
Data-parallel over 8 NeuronCores: each core owns a 64-row batch shard and the
full (replicated) weights.

v2 schedule (vs v1): the hidden dim is split into 4 slices of 256; each
slice's gates are matmul'd into its own pair of PSUM banks (4 slices x 2
banks = all 8), then drained by ACT/DVE while later slices' matmuls run.
The LSTM cell update runs per-slice:
  - grp0 (psum rows 0:64):  {i_s | g_s};  grp1 (rows 64:128): {f_s | o_s}
  - u = sig(i)*tanh(g) on rows 0:64, moved to rows 64:128 by a PE
    identity-matmul into a free region of the same psum tile
  - c (resident rows 64:128) updated in place; h = sig(o)*tanh(c) bf16
  - h slice is transposed [64,256]->[128,2,64] by the DMA xbar
    (dma_start_transpose) straight into the next step's stationary tiles
Steps ping-pong between two hT banks.  Emission is software-pipelined: the
last slice's cross-partition move and transpose are deferred into the next
step's instruction stream so the PE never waits on the cell-update chain,
and h6/h7 consumption waves are deferred past the producer transposes.
Matmul inputs are bf16; PSUM accumulation and c stay fp32.
"""

import sys
import contextlib

sys.path.insert(0, "/opt/trn_rl_repo")

import numpy as np

import concourse.bass as bass
import concourse.tile as tile
from concourse import bacc, mybir
from concourse.bass import ds
from concourse.bass_utils import run_bass_kernel_spmd

# Problem constants (hardcoded per contract)
N_FULL, L_FULL, I_DIM, H, O = 512, 512, 64, 1024, 1
N_CORES = 8
B = N_FULL // N_CORES        # 64 local batch rows
KX = I_DIM + 1               # x chunk contraction size (input + bias row)
NH = H // 128                # 8 hidden-dim chunks of 128
NK = NH + 1                  # total contraction chunks (x + 8 h chunks)
NS = 4                       # hidden slices per step
S = H // NS                  # 256 hidden per slice
U = 8                        # steps per dynamic-loop body (must be even)
XPAD = U                     # extra zero x rows so prefetch never reads OOB

f32 = mybir.dt.float32
bf16 = mybir.dt.bfloat16
AF = mybir.ActivationFunctionType


def build_program(L=L_FULL, force_static=False, xbar_split=True):
    """Build the per-core Bass program."""
    nc = bacc.Bacc("TRN2", target_bir_lowering=False, debug=False,
                   num_devices=N_CORES)

    # ---- DRAM I/O (per core) ----
    x_d = nc.dram_tensor("xT", [L + XPAD, KX, B], bf16,
                         kind="ExternalInput").ap()
    wg_d = nc.dram_tensor("Wg", [NK, 128, NS, 2, 2 * S], bf16,
                          kind="ExternalInput").ap()
    h0T_d = nc.dram_tensor("h0T", [NH, 128, B], bf16, kind="ExternalInput").ap()
    c0_d = nc.dram_tensor("c0", [B, H], f32, kind="ExternalInput").ap()
    id_d = nc.dram_tensor("idm", [B, B], bf16, kind="ExternalInput").ap()
    w1_d = nc.dram_tensor("W1p", [NK, 128, H], bf16, kind="ExternalInput").ap()
    w2_d = nc.dram_tensor("W2bc", [B, H], f32, kind="ExternalInput").ap()
    b2_d = nc.dram_tensor("b2bc", [B, 1], f32, kind="ExternalInput").ap()
    out_d = nc.dram_tensor("out", [B, 1], f32, kind="ExternalOutput").ap()

    with tile.TileContext(nc) as tc, contextlib.ExitStack() as ctx:
        singles = ctx.enter_context(tc.tile_pool(name="singles", bufs=1))
        xpool = ctx.enter_context(tc.tile_pool(name="xpool", bufs=3))
        actp = ctx.enter_context(tc.tile_pool(name="actp", bufs=3))
        w1pool = ctx.enter_context(tc.tile_pool(name="w1pool", bufs=2))
        gpsum = ctx.enter_context(tc.tile_pool(name="gpsum", bufs=1,
                                               space="PSUM"))

        # ---- resident SBUF state ----
        wg_sb = singles.tile([128, NK, NS, 2, 2 * S], bf16)
        nc.sync.dma_start(wg_sb[:], wg_d.rearrange("k p s g c -> p k s g c"))
        hT0 = singles.tile([128, NH, B], bf16)
        hT1 = singles.tile([128, NH, B], bf16)
        hT = [hT0, hT1]
        nc.sync.dma_start(hT[0][:], h0T_d.rearrange("j p b -> p j b"))
        cfull = singles.tile([128, H], f32)            # c on rows 64:128
        nc.sync.dma_start(cfull[B:128, :], c0_d)
        idT = singles.tile([B, B], bf16)               # identity for row moves
        nc.sync.dma_start(idT[:], id_d)
        ones1 = singles.tile([1, B], bf16)             # MLP bias stationary row
        nc.vector.memset(ones1[:], 1.0)

        def make_step(s_un, xt):
            """Closures for one step's emission (software-pipelined)."""
            par = s_un % 2
            hT_in, hT_out = hT[par], hT[1 - par]
            cur_ps = {}
            state = {}

            def pstile(s):
                if s not in cur_ps:
                    cur_ps[s] = gpsum.tile([128, 4 * S], f32, tag=f"ps{s}", name=f"ps{s}")
                return cur_ps[s]

            def stat(k):
                if k == 0:
                    return xt[:, :], KX
                return hT_in[:, k - 1, :], 128

            def waves(s, ks):
                p = pstile(s)
                for k in ks:
                    lhsT, kp = stat(k)
                    nc.tensor.matmul(
                        p[0:B, 0:2 * S], lhsT,
                        wg_sb[0:kp, k, s, 0, :],
                        start=(k == 0), stop=(k == NK - 1))
                    nc.tensor.matmul(
                        p[B:128, 2 * S:4 * S], lhsT,
                        wg_sb[0:kp, k, s, 1, :],
                        start=(k == 0), stop=(k == NK - 1))

            def drain(s):
                p = cur_ps[s]
                tg = actp.tile([B, S], bf16, tag="tg", name="tg")
                si = actp.tile([B, S], bf16, tag="si", name="si")
                fo = actp.tile([128, 2 * S], bf16, tag="fo", name="fo")
                nc.scalar.activation(tg[:], p[0:B, S:2 * S], AF.Tanh)
                nc.scalar.activation(si[:], p[0:B, 0:S], AF.Sigmoid)
                nc.scalar.activation(fo[B:128, :], p[B:128, 2 * S:4 * S],
                                     AF.Sigmoid)
                u = actp.tile([B, S], bf16, tag="u", name="u")
                nc.vector.tensor_mul(u[:], si[:], tg[:])
                state[s] = (u, fo)

            def imove(s):
                u, fo = state[s]
                nc.tensor.matmul(cur_ps[s][B:128, S:2 * S], idT[:], u[:],
                                 start=True, stop=True)

            def tail(s):
                u, fo = state[s]
                us = cur_ps[s][B:128, S:2 * S]
                ccol = cfull[B:128, s * S:(s + 1) * S]
                nc.vector.tensor_mul(ccol, fo[B:128, 0:S], ccol)
                nc.vector.tensor_add(ccol, ccol, us)
                tc_t = actp.tile([128, S], bf16, tag="tc", name="tc")
                nc.scalar.activation(tc_t[B:128, :], ccol, AF.Tanh)
                hsb = actp.tile([128, S], bf16, tag="h", name="hs")
                nc.vector.tensor_mul(hsb[B:128, :], fo[B:128, S:2 * S],
                                     tc_t[B:128, :])
                # DMA-xbar transpose straight into the stationary bank
                eng = nc.scalar if (xbar_split and s % 2) else nc.sync
                eng.dma_start(hT_out[:, 2 * s:2 * s + 2, :], hsb[B:128, :],
                              transpose=True)

            return waves, drain, imove, tail

        def emit_step(s_un, xt, prev):
            """Emit one step; `prev` holds the previous step's closures."""
            waves, drain, imove, tail = make_step(s_un, xt)
            waves(0, range(0, 7))            # k = x, h0..h5
            if prev is not None:
                prev[2](3)                   # imove(prev s3)
                prev[3](3)                   # tail(prev s3) + its transpose
            waves(1, range(0, 7))
            waves(0, range(7, NK))           # h6, h7
            waves(1, range(7, NK))
            drain(0)
            imove(0)
            waves(2, range(0, NK))
            tail(0)
            drain(1)
            imove(1)
            waves(3, range(0, NK))
            tail(1)
            drain(2)
            imove(2)
            tail(2)
            drain(3)                         # imove/tail of s3 deferred
            return (waves, drain, imove, tail)

        def flush(prev):
            if prev is not None:
                prev[2](3)
                prev[3](3)

        def load_x(idx):
            xt = xpool.tile([KX, B], bf16, tag="xt", name="xt")
            nc.sync.dma_start(xt[:], x_d[ds(idx, 1)].flatten_outer_dims())
            return xt

        if L % U == 0 and L > U and not force_static:
            with tc.For_i(0, L, U, hint_engines=(mybir.EngineType.PE,)) as iv0:
                prev = None
                pending = [load_x(iv0 + 0), load_x(iv0 + 1)]
                for s_un in range(U):
                    xt = pending.pop(0)
                    pending.append(load_x(iv0 + s_un + 2))
                    prev = emit_step(s_un, xt, prev)
                flush(prev)
        else:
            prev = None
            for t in range(L):
                prev = emit_step(t, load_x(t), prev)
            flush(prev)

        # ---- MLP head: out = sigmoid(h @ W1 + b1) @ W2 + b2 ----
        par_end = L % 2
        hT_fin = hT[par_end]
        zt = gpsum.tile([128, 4 * S], f32, tag="ps0", name="zps")
        zps = zt[0:B, 0:H]
        for k in range(NK):
            w1t = w1pool.tile([128, H], bf16, tag="w1", name="w1t")
            nc.sync.dma_start(w1t[:], w1_d[k])
            if k < NH:
                lhsT, kp = hT_fin[:, k, :], 128
            else:
                lhsT, kp = ones1[:, :], 1
            for hh in range(2):
                nc.tensor.matmul(
                    zps[:, hh * 512:(hh + 1) * 512],
                    lhsT, w1t[0:kp, hh * 512:(hh + 1) * 512],
                    start=(k == 0), stop=(k == NK - 1))
        z_sb = actp.tile([B, H], f32, tag="z")
        nc.scalar.activation(z_sb[:], zps, AF.Sigmoid)
        w2_sb = actp.tile([B, H], f32, tag="w2")
        nc.sync.dma_start(w2_sb[:], w2_d)
        nc.vector.tensor_mul(z_sb[:], z_sb[:], w2_sb[:])
        red = actp.tile([B, 1], f32, tag="red")
        nc.vector.reduce_sum(red[:], z_sb[:], axis=mybir.AxisListType.X)
        b2_sb = actp.tile([B, 1], f32, tag="b2")
        nc.sync.dma_start(b2_sb[:], b2_d)
        nc.vector.tensor_add(red[:], red[:], b2_sb[:])
        nc.sync.dma_start(out_d[:], red[:])

    nc.compile()
    return nc


def prep_inputs(x, c, h, Wx, Wh, b, W1, b1, W2, b2, L=L_FULL):
    """Shard + lay out inputs for the 8 cores. Returns list of in_maps."""
    import ml_dtypes

    st_np = ml_dtypes.bfloat16

    x = np.asarray(x, np.float32)
    c = np.asarray(c, np.float32)
    h = np.asarray(h, np.float32)
    Wx = np.asarray(Wx, np.float32)
    Wh = np.asarray(Wh, np.float32)
    b = np.asarray(b, np.float32)
    W1 = np.asarray(W1, np.float32)
    b1 = np.asarray(b1, np.float32)
    W2 = np.asarray(W2, np.float32)
    b2 = np.asarray(b2, np.float32)

    # gate-weight tensor: [k, 128, slice, grp, 2S]
    # grp0 = [Wi_s | Wg_s], grp1 = [Wf_s | Wo_s]   (reference order i,f,g,o)
    W4 = np.concatenate([Wx, b[None, :], Wh], axis=0)        # [1089, 4H]
    Wg = np.zeros((NK, 128, NS, 2, 2 * S), np.float32)
    for k in range(NK):
        rows = W4[0:KX] if k == 0 else W4[KX + 128 * (k - 1):KX + 128 * k]
        rr = rows.reshape(-1, 4, H)
        for s in range(NS):
            cs = slice(s * S, (s + 1) * S)
            Wg[k, :rr.shape[0], s, 0, 0:S] = rr[:, 0, cs]      # i
            Wg[k, :rr.shape[0], s, 0, S:2 * S] = rr[:, 2, cs]  # g
            Wg[k, :rr.shape[0], s, 1, 0:S] = rr[:, 1, cs]      # f
            Wg[k, :rr.shape[0], s, 1, S:2 * S] = rr[:, 3, cs]  # o
    Wg = Wg.astype(st_np)

    W1p = np.zeros((NK, 128, H), np.float32)
    W1p[:NH] = W1.reshape(NH, 128, H)
    W1p[NH, 0] = b1
    W1p = W1p.astype(st_np)

    idm = np.eye(B, dtype=np.float32).astype(st_np)

    in_maps = []
    for cix in range(N_CORES):
        sl = slice(cix * B, (cix + 1) * B)
        xc = x[sl, :L, :]                                     # [B, L, I]
        xT = np.concatenate(
            [xc.transpose(1, 2, 0), np.ones((L, 1, B), np.float32)], axis=1
        )                                                     # [L, I+1, B]
        xT = np.concatenate(
            [xT, np.zeros((XPAD, KX, B), np.float32)], axis=0)
        h0T = h[sl].T.reshape(NH, 128, B)                     # [NH, 128, B]
        in_maps.append({
            "xT": np.ascontiguousarray(xT).astype(st_np),
            "Wg": Wg,
            "h0T": np.ascontiguousarray(h0T).astype(st_np),
            "c0": np.ascontiguousarray(c[sl]),
            "idm": idm,
            "W1p": W1p,
            "W2bc": np.ascontiguousarray(
                np.broadcast_to(W2[:, 0][None, :], (B, H))),
            "b2bc": np.full((B, 1), np.float32(b2[0])),
        })
    return in_maps


_CACHED_NC = None


def kernel(**inputs) -> np.ndarray:
    global _CACHED_NC
    if _CACHED_NC is None:
        _CACHED_NC = build_program()
    in_maps = prep_inputs(**inputs)
    res = run_bass_kernel_spmd(_CACHED_NC, in_maps, core_ids=list(range(N_CORES)))
    out = np.concatenate([res.results[cix]["out"][:, 0] for cix in range(N_CORES)])
    return out.astype(np.float32)


if __name__ == "__main__":
    print("kernel.py loaded OK")
